# revision 11
# baseline (speedup 1.0000x reference)
"""Trainium2 Bass kernel for batched multi-mask masked-mean (segment_reduce).

Computes, for each (batch, area) pair and each of two mask tensors:
    m   = smooth-AND over 4 channels of differentiable_eq(mask, initial_mask_id)
    out = m * (sum(m * img) / sum(m))        (masked mean over the 16x16 patch)

Sharding: data-parallel over the flattened (batch * n_areas) axis across 8
NeuronCores; no cross-core communication.

Math notes:
  diff_round(x) = x - sin(2*pi*x)/(2*pi).  Work in "y-space" (y = 2*pi*x):
  f(y) = y - sin(y); harder_diff_round(x) = f(f(f(2*pi*x)))/(2*pi).
  The ScalarEngine Sin spline is valid only on [-pi, pi], so every sin(y) for
  y in [0, 2*pi] is computed as -sin(y - pi) via the activation's free affine
  (bias = -pi), turning all f-step subtracts into adds.
  differentiable_eq(a, B) with B = hdr(id) constant per (area, channel) is the
  affine  t = A*(2B-1) + (1-B)  of A = hdr(a); in y-space z = yA*S + U with
  S = 2B-1, U = 2*pi*(1-B), both precomputed on host (tiny).
  The masked mean is scale-invariant in m, so the pipeline carries
  m~ = (2*pi)^2 * m and only rescales in the final per-area multiply.
"""

import numpy as np

import concourse.bacc as bacc
import concourse.mybir as mybir
import concourse.tile as tile
from concourse.bass_utils import run_bass_kernel_spmd

# ---------------------------------------------------------------- geometry
N_CORES = 8
B, N, DX, DY, C = 2, 8192, 16, 16, 4
PIX = DX * DY                      # 256 pixels per area
W_IN = PIX * C                     # 1024 mask values per area (channel-interleaved)
A_TOT = B * N                      # 16384 areas
A_CORE = A_TOT // N_CORES          # 2048 areas per core
P = 128                            # SBUF partitions

PI = float(np.pi)
TWO_PI = float(2.0 * np.pi)
EPS_GUARD = 2e-5                   # keeps sin args strictly inside [-pi, pi]
GA = 1.0 - EPS_GUARD
INV_4PI2 = float(1.0 / (4.0 * np.pi * np.pi))

F32 = mybir.dt.float32
BF16 = mybir.dt.bfloat16
SIN = mybir.ActivationFunctionType.Sin
COPY = mybir.ActivationFunctionType.Copy
MULT = mybir.AluOpType.mult
ADD = mybir.AluOpType.add
AX_X = mybir.AxisListType.X

# compute dtype for the bulk elementwise pipeline ("f32" or "bf16")
COMPUTE = "f32"
G = 2                              # areas per partition per mega-tile
# engine for each f-step add (DVE is ~2x faster than GPSIMD per op, but they
# run concurrently; alternating spreads the serial chain across both)
ADD_ENGINES = ("vector", "gpsimd", "vector", "gpsimd", "vector", "gpsimd", "vector")
FUSE_AB = True                     # one [2Q] sin+stt for the a/b pair
FUSE_ACCUM = True                  # stt accum_out for den/num reductions


def build(nc, a_core=A_CORE, g=G, compute=COMPUTE, add_engines=ADD_ENGINES,
          fuse_ab=FUSE_AB, fuse_accum=FUSE_ACCUM):
    """Emit the Tile graph onto `nc` for one core's shard of `a_core` areas."""
    dt = F32 if compute == "f32" else BF16
    W = g * W_IN                   # mega-tile mask width (f32 elems per partition)
    Q = g * PIX                    # mega-tile single-channel width
    n_tiles = a_core // (P * g)
    assert n_tiles * P * g == a_core

    d_mask = nc.dram_tensor("mask", [a_core, W_IN], F32, kind="ExternalInput")
    d_alt = nc.dram_tensor("alt", [a_core, W_IN], F32, kind="ExternalInput")
    d_img = nc.dram_tensor("img", [a_core, PIX], F32, kind="ExternalInput")
    d_su = nc.dram_tensor("su", [a_core, 8], F32, kind="ExternalInput")
    d_out = nc.dram_tensor("out", [a_core, PIX], F32, kind="ExternalOutput")
    d_outa = nc.dram_tensor("outalt", [a_core, PIX], F32, kind="ExternalOutput")

    mask_v = d_mask.ap().rearrange("(t p g) f -> t p (g f)", p=P, g=g)
    alt_v = d_alt.ap().rearrange("(t p g) f -> t p (g f)", p=P, g=g)
    img_v = d_img.ap().rearrange("(t p g) f -> t p (g f)", p=P, g=g)
    su_v = d_su.ap().rearrange("(t p g) c -> p t g c", p=P, g=g)
    out_v = d_out.ap().rearrange("(t p g) f -> t p (g f)", p=P, g=g)
    outa_v = d_outa.ap().rearrange("(t p g) f -> t p (g f)", p=P, g=g)

    with tile.TileContext(nc) as tc:
        from contextlib import ExitStack

        with ExitStack() as ctx:
            const = ctx.enter_context(tc.tile_pool(name="const", bufs=1))
            big = ctx.enter_context(tc.tile_pool(name="big", bufs=4))
            med = ctx.enter_context(tc.tile_pool(name="med", bufs=2))
            sm = ctx.enter_context(tc.tile_pool(name="sm", bufs=2))

            nb = const.tile([P, 1], F32, tag="nb")       # -pi*GA bias for sin
            nc.gpsimd.memset(nb[:], -PI * GA)
            su_sb = const.tile([P, n_tiles * g * 8], F32, tag="su")
            nc.sync.dma_start(
                su_sb[:].rearrange("p (t g c) -> p t g c", t=n_tiles, g=g), su_v
            )

            def f_step(y, width, tag, engine="vector"):
                """y <- f(y) = y - sin(y), via s = -sin(y) then add."""
                s = big.tile([P, width], dt, tag="sin")
                nc.scalar.activation(s[:], y[:], SIN, scale=GA, bias=nb[:])
                y2 = big.tile([P, width], dt, tag=tag)
                eng = nc.vector if engine == "vector" else nc.gpsimd
                eng.tensor_tensor(y2[:], y[:], s[:], ADD)
                return y2

            for t in range(n_tiles):
                img_sb = med.tile([P, Q], F32, tag="img")
                nc.sync.dma_start(img_sb[:], img_v[t])
                if compute != "f32":
                    img_c = med.tile([P, Q], dt, tag="imgc")
                    nc.vector.tensor_copy(img_c[:], img_sb[:])
                else:
                    img_c = img_sb

                for j, (src_v, dst_v) in enumerate(
                    ((mask_v, out_v), (alt_v, outa_v))
                ):
                    x = big.tile([P, W], F32, tag="x")
                    nc.sync.dma_start(x[:], src_v[t])

                    # ---- A phase: y3 = f^3(2*pi*x)  (hdr of mask, y-space)
                    s0 = big.tile([P, W], dt, tag="sin")
                    nc.scalar.activation(
                        s0[:], x[:], SIN, scale=TWO_PI * GA, bias=nb[:]
                    )
                    y1 = big.tile([P, W], dt, tag="yy")
                    eng0 = nc.vector if add_engines[0] == "vector" else nc.gpsimd
                    if compute == "f32":
                        eng0.scalar_tensor_tensor(
                            y1[:], x[:], TWO_PI, s0[:], MULT, ADD
                        )
                    else:
                        y0 = big.tile([P, W], dt, tag="y0")
                        nc.scalar.activation(y0[:], x[:], COPY, scale=TWO_PI)
                        eng0.tensor_tensor(y1[:], y0[:], s0[:], ADD)
                    y2 = f_step(y1, W, "yy", add_engines[1])
                    y3 = f_step(y2, W, "yy", add_engines[2])

                    # ---- eq phase: z = y3*S + U per (area, channel),
                    # de-interleaving to channel-major [c][g][pix] layout
                    z = big.tile([P, W], dt, tag="zz")
                    y3v = y3[:].rearrange("p (g i c) -> p g c i", g=g, c=C)
                    zv = z[:].rearrange("p (c g i) -> p c g i", c=C, g=g)
                    for gg in range(g):
                        col = (t * g + gg) * 8
                        for c in range(C):
                            nc.vector.tensor_scalar(
                                zv[:, c, gg, :],
                                y3v[:, gg, c, :],
                                su_sb[:, col + c : col + c + 1],
                                su_sb[:, col + 4 + c : col + 4 + c + 1],
                                MULT,
                                ADD,
                            )
                    # f^3 -> e (y-space eq), then w = f(e) = 2*pi*dr(eq)
                    e1 = f_step(z, W, "zz", add_engines[3])
                    e2 = f_step(e1, W, "zz", add_engines[4])
                    e3 = f_step(e2, W, "zz", add_engines[5])
                    w = f_step(e3, W, "zz", add_engines[6])

                    # ---- AND phase (channel-major blocks are contiguous)
                    if fuse_ab:
                        ab = med.tile([P, 2 * Q], dt, tag="ab")
                        nc.vector.tensor_tensor(
                            ab[:, 0:Q], w[:, 0:Q], w[:, Q : 2 * Q], MULT
                        )
                        nc.vector.tensor_tensor(
                            ab[:, Q : 2 * Q],
                            w[:, 2 * Q : 3 * Q],
                            w[:, 3 * Q : 4 * Q],
                            MULT,
                        )
                        sab = med.tile([P, 2 * Q], dt, tag="sab")
                        nc.scalar.activation(
                            sab[:], ab[:], SIN, scale=GA / TWO_PI, bias=nb[:]
                        )
                        fab = med.tile([P, 2 * Q], dt, tag="fab")
                        nc.vector.scalar_tensor_tensor(
                            fab[:], ab[:], 1.0 / TWO_PI, sab[:], MULT, ADD
                        )
                        fa, fb = fab[:, 0:Q], fab[:, Q : 2 * Q]
                    else:
                        a = med.tile([P, Q], dt, tag="aa")
                        nc.vector.tensor_tensor(
                            a[:], w[:, 0:Q], w[:, Q : 2 * Q], MULT
                        )
                        b = med.tile([P, Q], dt, tag="bb")
                        nc.vector.tensor_tensor(
                            b[:], w[:, 2 * Q : 3 * Q], w[:, 3 * Q : 4 * Q], MULT
                        )
                        sa = med.tile([P, Q], dt, tag="sa")
                        nc.scalar.activation(
                            sa[:], a[:], SIN, scale=GA / TWO_PI, bias=nb[:]
                        )
                        fa = med.tile([P, Q], dt, tag="fa")
                        nc.vector.scalar_tensor_tensor(
                            fa[:], a[:], 1.0 / TWO_PI, sa[:], MULT, ADD
                        )
                        sb_ = med.tile([P, Q], dt, tag="sb")
                        nc.scalar.activation(
                            sb_[:], b[:], SIN, scale=GA / TWO_PI, bias=nb[:]
                        )
                        fb = med.tile([P, Q], dt, tag="fb")
                        nc.vector.scalar_tensor_tensor(
                            fb[:], b[:], 1.0 / TWO_PI, sb_[:], MULT, ADD
                        )

                    den = sm.tile([P, g], F32, tag="den")
                    num = sm.tile([P, g], F32, tag="num")
                    m = med.tile([P, Q], dt, tag="mm")
                    if fuse_accum:
                        mi = med.tile([P, Q], dt, tag="mi")
                        for gg in range(g):
                            gs = slice(gg * PIX, (gg + 1) * PIX)
                            nc.vector.scalar_tensor_tensor(
                                m[:, gs], fa[:, gs], 0.0, fb[:, gs],
                                mybir.AluOpType.bypass, MULT,
                                accum_out=den[:, gg : gg + 1],
                            )
                            nc.vector.scalar_tensor_tensor(
                                mi[:, gs], m[:, gs], 0.0, img_c[:, gs],
                                mybir.AluOpType.bypass, MULT,
                                accum_out=num[:, gg : gg + 1],
                            )
                    else:
                        nc.vector.tensor_tensor(m[:], fa[:], fb[:], MULT)
                        mi = med.tile([P, Q], dt, tag="mi")
                        nc.vector.tensor_tensor(mi[:], m[:], img_c[:], MULT)
                        nc.vector.tensor_reduce(
                            den[:], m[:].rearrange("p (g i) -> p g i", g=g),
                            AX_X, ADD,
                        )
                        nc.vector.tensor_reduce(
                            num[:], mi[:].rearrange("p (g i) -> p g i", g=g),
                            AX_X, ADD,
                        )
                    rd = sm.tile([P, g], F32, tag="rd")
                    nc.vector.reciprocal(rd[:], den[:])
                    q = sm.tile([P, g], F32, tag="qq")
                    nc.vector.tensor_tensor(q[:], num[:], rd[:], MULT)

                    o = med.tile([P, Q], F32, tag="oo")
                    for gg in range(g):
                        nc.vector.tensor_scalar(
                            o[:, gg * PIX : (gg + 1) * PIX],
                            m[:, gg * PIX : (gg + 1) * PIX],
                            q[:, gg : gg + 1],
                            INV_4PI2,
                            MULT,
                            MULT,
                        )
                    nc.sync.dma_start(dst_v[t], o[:])

    return nc


# ------------------------------------------------------------- host helpers
def _hdr_np(x):
    def dr(v):
        return v - np.sin(2.0 * np.pi * v) / (2.0 * np.pi)

    return dr(dr(dr(x)))


_NC_CACHE = {}


def _get_compiled():
    key = (COMPUTE, G)
    if key not in _NC_CACHE:
        nc = bacc.Bacc(
            "TRN2", target_bir_lowering=False, debug=False, num_devices=N_CORES
        )
        build(nc, A_CORE, G, COMPUTE)
        nc.compile()
        _NC_CACHE[key] = nc
    return _NC_CACHE[key]


def _make_in_maps(resized_image, mask_combined, mask_combined_alt, initial_mask_id):
    mask = np.ascontiguousarray(
        np.asarray(mask_combined, dtype=np.float32).reshape(A_TOT, W_IN)
    )
    alt = np.ascontiguousarray(
        np.asarray(mask_combined_alt, dtype=np.float32).reshape(A_TOT, W_IN)
    )
    img = np.ascontiguousarray(
        np.asarray(resized_image, dtype=np.float32).reshape(A_TOT, PIX)
    )
    idf = np.asarray(initial_mask_id, dtype=np.float64).reshape(A_TOT, C)
    bh = _hdr_np(idf)
    su = np.concatenate(
        [2.0 * bh - 1.0, 2.0 * np.pi * (1.0 - bh)], axis=1
    ).astype(np.float32)

    in_maps = []
    for k in range(N_CORES):
        sl = slice(k * A_CORE, (k + 1) * A_CORE)
        in_maps.append(
            {"mask": mask[sl], "alt": alt[sl], "img": img[sl], "su": su[sl]}
        )
    return in_maps


def run(inputs, trace=False, trace_kwargs=None):
    """Run the kernel on all 8 cores; returns ((out, out_alt), exec_time_ns)."""
    nc = _get_compiled()
    in_maps = _make_in_maps(
        inputs["resized_image"],
        inputs["mask_combined"],
        inputs["mask_combined_alt"],
        inputs["initial_mask_id"],
    )
    res = run_bass_kernel_spmd(
        nc,
        in_maps,
        list(range(N_CORES)),
        trace=trace,
        **(trace_kwargs or {}),
    )
    out = np.empty((A_TOT, PIX), np.float32)
    outa = np.empty((A_TOT, PIX), np.float32)
    for k in range(N_CORES):
        sl = slice(k * A_CORE, (k + 1) * A_CORE)
        out[sl] = res.results[k]["out"]
        outa[sl] = res.results[k]["outalt"]
    shape = (B, N, DX, DY, 1)
    return (out.reshape(shape), outa.reshape(shape)), res.exec_time_ns


def kernel(**inputs):
    (out, outa), _ = run(inputs, trace=False)
    return out, outa


# revision 12
# speedup vs baseline: 1.5579x; 1.5579x over previous
"""Trainium2 Bass kernel for batched multi-mask masked-mean (segment_reduce).

Computes, for each (batch, area) pair and each of two mask tensors:
    m   = smooth-AND over 4 channels of differentiable_eq(mask, initial_mask_id)
    out = m * (sum(m * img) / sum(m))        (masked mean over the 16x16 patch)

Sharding: data-parallel over the flattened (batch * n_areas) axis across 8
NeuronCores; no cross-core communication.

Math notes:
  diff_round(x) = x - sin(2*pi*x)/(2*pi).  Work in "y-space" (y = 2*pi*x):
  f(y) = y - sin(y); harder_diff_round(x) = f(f(f(2*pi*x)))/(2*pi).
  The ScalarEngine Sin spline is valid only on [-pi, pi], so every sin(y) for
  y in [0, 2*pi] is computed as -sin(y - pi) via the activation's free affine
  (bias = -pi), turning all f-step subtracts into adds.
  differentiable_eq(a, B) with B = hdr(id) constant per (area, channel) is the
  affine  t = A*(2B-1) + (1-B)  of A = hdr(a); in y-space z = yA*S + U with
  S = 2B-1, U = 2*pi*(1-B), both precomputed on host (tiny).
  The masked mean is scale-invariant in m, so the pipeline carries
  m~ = (2*pi)^2 * m and only rescales in the final per-area multiply.
"""

import itertools

import numpy as np

import concourse.bacc as bacc
import concourse.mybir as mybir
import concourse.tile as tile
from concourse.bass_utils import run_bass_kernel_spmd

# ---------------------------------------------------------------- geometry
N_CORES = 8
B, N, DX, DY, C = 2, 8192, 16, 16, 4
PIX = DX * DY                      # 256 pixels per area
W_IN = PIX * C                     # 1024 mask values per area (channel-interleaved)
A_TOT = B * N                      # 16384 areas
A_CORE = A_TOT // N_CORES          # 2048 areas per core
P = 128                            # SBUF partitions

PI = float(np.pi)
TWO_PI = float(2.0 * np.pi)
EPS_GUARD = 2e-5                   # keeps sin args strictly inside [-pi, pi]
GA = 1.0 - EPS_GUARD
INV_4PI2 = float(1.0 / (4.0 * np.pi * np.pi))

F32 = mybir.dt.float32
BF16 = mybir.dt.bfloat16
SIN = mybir.ActivationFunctionType.Sin
COPY = mybir.ActivationFunctionType.Copy
MULT = mybir.AluOpType.mult
ADD = mybir.AluOpType.add
BYPASS = mybir.AluOpType.bypass
AX_X = mybir.AxisListType.X

# compute dtype for the bulk elementwise pipeline ("f32" or "bf16")
COMPUTE = "f32"
G = 2                              # areas per partition per mega-tile
BIG_BUFS = 4
MED_BUFS = 3


def build(nc, a_core=A_CORE, g=G, compute=COMPUTE):
    """Emit the Tile graph onto `nc` for one core's shard of `a_core` areas."""
    dt = F32 if compute == "f32" else BF16
    W = g * W_IN                   # mega-tile mask width (f32 elems per partition)
    Q = g * PIX                    # mega-tile single-channel width
    n_tiles = a_core // (P * g)
    assert n_tiles * P * g == a_core

    d_mask = nc.dram_tensor("mask", [a_core, W_IN], F32, kind="ExternalInput")
    d_alt = nc.dram_tensor("alt", [a_core, W_IN], F32, kind="ExternalInput")
    d_img = nc.dram_tensor("img", [a_core, PIX], F32, kind="ExternalInput")
    d_su = nc.dram_tensor("su", [a_core, 8], F32, kind="ExternalInput")
    d_out = nc.dram_tensor("out", [a_core, PIX], F32, kind="ExternalOutput")
    d_outa = nc.dram_tensor("outalt", [a_core, PIX], F32, kind="ExternalOutput")

    mask_v = d_mask.ap().rearrange("(t p g) f -> t p (g f)", p=P, g=g)
    alt_v = d_alt.ap().rearrange("(t p g) f -> t p (g f)", p=P, g=g)
    img_v = d_img.ap().rearrange("(t p g) f -> t p (g f)", p=P, g=g)
    su_v = d_su.ap().rearrange("(t p g) c -> p t g c", p=P, g=g)
    out_v = d_out.ap().rearrange("(t p g) f -> t p (g f)", p=P, g=g)
    outa_v = d_outa.ap().rearrange("(t p g) f -> t p (g f)", p=P, g=g)

    with tile.TileContext(nc) as tc:
        from contextlib import ExitStack

        with ExitStack() as ctx:
            const = ctx.enter_context(tc.tile_pool(name="const", bufs=1))
            big = ctx.enter_context(tc.tile_pool(name="big", bufs=BIG_BUFS))
            med = ctx.enter_context(tc.tile_pool(name="med", bufs=MED_BUFS))
            sm = ctx.enter_context(tc.tile_pool(name="sm", bufs=MED_BUFS))

            nb = const.tile([P, 1], F32, tag="nb")       # -pi*GA bias for sin
            nc.gpsimd.memset(nb[:], -PI * GA)
            su_sb = const.tile([P, n_tiles * g * 8], F32, tag="su")
            nc.sync.dma_start(
                su_sb[:].rearrange("p (t g c) -> p t g c", t=n_tiles, g=g), su_v
            )

            def f_step(y, tag):
                """y <- f(y) = y - sin(y), via s = -sin(y) then add."""
                s = big.tile([P, W], dt, tag="sin")
                nc.scalar.activation(s[:], y[:], SIN, scale=GA, bias=nb[:])
                y2 = big.tile([P, W], dt, tag=tag)
                nc.vector.tensor_tensor(y2[:], y[:], s[:], ADD)
                return y2

            def emit_pass(t, j, img_c):
                src_v, dst_v = ((mask_v, out_v), (alt_v, outa_v))[j]
                x = big.tile([P, W], F32, tag="x")
                nc.sync.dma_start(x[:], src_v[t])

                # ---- A phase: y3 = f^3(2*pi*x)  (hdr of mask, y-space)
                s0 = big.tile([P, W], dt, tag="sin")
                nc.scalar.activation(s0[:], x[:], SIN, scale=TWO_PI * GA, bias=nb[:])
                y1 = big.tile([P, W], dt, tag="yy")
                if compute == "f32":
                    nc.vector.scalar_tensor_tensor(
                        y1[:], x[:], TWO_PI, s0[:], MULT, ADD
                    )
                else:
                    y0 = big.tile([P, W], dt, tag="y0")
                    nc.scalar.activation(y0[:], x[:], COPY, scale=TWO_PI)
                    nc.vector.tensor_tensor(y1[:], y0[:], s0[:], ADD)
                y2 = f_step(y1, "yy")
                y3 = f_step(y2, "yy")
                yield

                # ---- eq phase: z = y3*S + U per (area, channel),
                # de-interleaving to channel-major [c][g][pix] layout
                z = big.tile([P, W], dt, tag="zz")
                y3v = y3[:].rearrange("p (g i c) -> p g c i", g=g, c=C)
                zv = z[:].rearrange("p (c g i) -> p c g i", c=C, g=g)
                for gg in range(g):
                    col = (t * g + gg) * 8
                    for c in range(C):
                        nc.vector.tensor_scalar(
                            zv[:, c, gg, :],
                            y3v[:, gg, c, :],
                            su_sb[:, col + c : col + c + 1],
                            su_sb[:, col + 4 + c : col + 4 + c + 1],
                            MULT,
                            ADD,
                        )
                # f^3 -> e (y-space eq), then w = f(e) = 2*pi*dr(eq)
                e1 = f_step(z, "zz")
                e2 = f_step(e1, "zz")
                e3 = f_step(e2, "zz")
                w = f_step(e3, "zz")
                yield

                # ---- AND phase (channel-major blocks are contiguous)
                ab = med.tile([P, 2 * Q], dt, tag="ab")
                nc.vector.tensor_tensor(ab[:, 0:Q], w[:, 0:Q], w[:, Q : 2 * Q], MULT)
                nc.vector.tensor_tensor(
                    ab[:, Q : 2 * Q], w[:, 2 * Q : 3 * Q], w[:, 3 * Q : 4 * Q], MULT
                )
                sab = med.tile([P, 2 * Q], dt, tag="sab")
                nc.scalar.activation(sab[:], ab[:], SIN, scale=GA / TWO_PI, bias=nb[:])
                fab = med.tile([P, 2 * Q], dt, tag="fab")
                nc.vector.scalar_tensor_tensor(
                    fab[:], ab[:], 1.0 / TWO_PI, sab[:], MULT, ADD
                )
                fa, fb = fab[:, 0:Q], fab[:, Q : 2 * Q]

                den = sm.tile([P, g], F32, tag="den")
                num = sm.tile([P, g], F32, tag="num")
                m = med.tile([P, Q], dt, tag="mm")
                mi = med.tile([P, Q], dt, tag="mi")
                for gg in range(g):
                    gs = slice(gg * PIX, (gg + 1) * PIX)
                    nc.vector.scalar_tensor_tensor(
                        m[:, gs], fa[:, gs], 0.0, fb[:, gs], BYPASS, MULT,
                        accum_out=den[:, gg : gg + 1],
                    )
                    nc.vector.scalar_tensor_tensor(
                        mi[:, gs], m[:, gs], 0.0, img_c[:, gs], BYPASS, MULT,
                        accum_out=num[:, gg : gg + 1],
                    )
                rd = sm.tile([P, g], F32, tag="rd")
                nc.vector.reciprocal(rd[:], den[:])
                q = sm.tile([P, g], F32, tag="qq")
                nc.vector.tensor_tensor(q[:], num[:], rd[:], MULT)

                o = med.tile([P, Q], F32, tag="oo")
                for gg in range(g):
                    nc.vector.tensor_scalar(
                        o[:, gg * PIX : (gg + 1) * PIX],
                        m[:, gg * PIX : (gg + 1) * PIX],
                        q[:, gg : gg + 1],
                        INV_4PI2,
                        MULT,
                        MULT,
                    )
                nc.sync.dma_start(dst_v[t], o[:])
                yield

            for t in range(n_tiles):
                img_sb = med.tile([P, Q], F32, tag="img")
                nc.sync.dma_start(img_sb[:], img_v[t])
                if compute != "f32":
                    img_c = med.tile([P, Q], dt, tag="imgc")
                    nc.vector.tensor_copy(img_c[:], img_sb[:])
                else:
                    img_c = img_sb
                # interleave the two independent mask pipelines phase-by-phase
                for _ in itertools.zip_longest(
                    emit_pass(t, 0, img_c), emit_pass(t, 1, img_c)
                ):
                    pass

    return nc


# ------------------------------------------------------------- host helpers
def _hdr_np(x):
    def dr(v):
        return v - np.sin(2.0 * np.pi * v) / (2.0 * np.pi)

    return dr(dr(dr(x)))


_NC_CACHE = {}


def _get_compiled():
    key = (COMPUTE, G)
    if key not in _NC_CACHE:
        nc = bacc.Bacc(
            "TRN2", target_bir_lowering=False, debug=False, num_devices=N_CORES
        )
        build(nc, A_CORE, G, COMPUTE)
        nc.compile()
        _NC_CACHE[key] = nc
    return _NC_CACHE[key]


def _make_in_maps(resized_image, mask_combined, mask_combined_alt, initial_mask_id):
    mask = np.ascontiguousarray(
        np.asarray(mask_combined, dtype=np.float32).reshape(A_TOT, W_IN)
    )
    alt = np.ascontiguousarray(
        np.asarray(mask_combined_alt, dtype=np.float32).reshape(A_TOT, W_IN)
    )
    img = np.ascontiguousarray(
        np.asarray(resized_image, dtype=np.float32).reshape(A_TOT, PIX)
    )
    idf = np.asarray(initial_mask_id, dtype=np.float64).reshape(A_TOT, C)
    bh = _hdr_np(idf)
    su = np.concatenate(
        [2.0 * bh - 1.0, 2.0 * np.pi * (1.0 - bh)], axis=1
    ).astype(np.float32)

    in_maps = []
    for k in range(N_CORES):
        sl = slice(k * A_CORE, (k + 1) * A_CORE)
        in_maps.append(
            {"mask": mask[sl], "alt": alt[sl], "img": img[sl], "su": su[sl]}
        )
    return in_maps


def run(inputs, trace=False, trace_kwargs=None):
    """Run the kernel on all 8 cores; returns ((out, out_alt), exec_time_ns)."""
    nc = _get_compiled()
    in_maps = _make_in_maps(
        inputs["resized_image"],
        inputs["mask_combined"],
        inputs["mask_combined_alt"],
        inputs["initial_mask_id"],
    )
    res = run_bass_kernel_spmd(
        nc,
        in_maps,
        list(range(N_CORES)),
        trace=trace,
        **(trace_kwargs or {}),
    )
    out = np.empty((A_TOT, PIX), np.float32)
    outa = np.empty((A_TOT, PIX), np.float32)
    for k in range(N_CORES):
        sl = slice(k * A_CORE, (k + 1) * A_CORE)
        out[sl] = res.results[k]["out"]
        outa[sl] = res.results[k]["outalt"]
    shape = (B, N, DX, DY, 1)
    return (out.reshape(shape), outa.reshape(shape)), res.exec_time_ns


def kernel(**inputs):
    (out, outa), _ = run(inputs, trace=False)
    return out, outa
